# revision 28
# baseline (speedup 1.0000x reference)
"""IF spiking-neuron scan (charge / fire / hard-reset) on 8 Trainium2 cores.

Reference recurrence over t (elementwise on every [B, N] element):
    v = v + x_t
    s = (v - 1.0 >= 0)          # spike, 0.0/1.0
    v = (1 - s) * v             # hard reset to 0

Sharding: pure data parallel over the B*N = 262144 element chains;
each of the 8 cores owns 32768 chains with zero communication.
Per core the chains live in SBUF as 128 partitions x 256 columns; the
64-step scan runs locally, bit-exact vs the reference.

v3 design (custom fused DVE op; ~46 us vs the 66.7 us v2, bit-exact):
  - The recurrence is restated on the pre-reset potential u_t = v_t + x_t:
        u_t = select(u_{t-1} < 1, u_{t-1}, 0) + x_t
    and a runtime-registered custom DVE uop (IF_STEP_ANT, 3 ALU stages)
    computes that in ONE Vector instruction per step per stream (v2
    needed two: tensor_add + fused scalar_tensor_tensor). Registration
    uses the standard dve_ops extension point; the uop table is written
    per-NEFF, no firmware change. Measured per-op cost matches the
    (ncols+151)/0.96 ns fp32 model; two interleaved half-width streams
    issue every ~202 ns (measured), beating one full-width stream
    (460 ns/step) because the dependent-op latency is hidden. 2x_2p
    perf mode is impossible here: it needs both SBUF read ports for one
    tensor, and the op reads two tensors.
  - The final step is IF_LAST_ANT: z_63 = [reset(u_62)+x_63 < 1] written
    straight to the uint8 spike buffer - no u_63 store, no tail z pass.
  - Other spikes are ONE ACT pass per block: z = Sign(V_TH - u) written
    directly as uint8. The float->uint8 conversion saturates negatives
    to 0, so z = [u < V_TH] = 1 - s exactly, including u == V_TH
    (Sign(0) = 0 -> spike). The host flips it back. The last blocks
    compute z on the DVE instead (tensor_scalar is_lt) as a drain tail.
  - Input x is pre-transposed on the host to [P, T, F] per core, so a
    timestep chunk is ONE contiguous multi-KiB descriptor per partition.
    Each 8-step chunk is split between the SP (HWDGE) and GpSimd (SWDGE)
    rings; all input dma_starts are hoisted before the compute so the
    in-order sequencers queue them up front. 16-step chunks and
    whole-chunk ring alternation both measured worse (lumpy delivery
    stalls the chain, which consumes 128 KiB every ~404 ns).
  - Outputs ride the ACT ring (which carries no input), merged into ~8
    grouped transfers so they stream during the chain; each extra DMA
    costs a sem the fixed epilogue pays for. The epilogue itself
    (every engine zeroes the full 250-sem space one by one, ~9 us) and
    the ~5 us startup are framework-fixed: confirmed invariant across
    29..48 dma_start kernels.
  - The whole input stays resident in SBUF; the u history is ONE flat
    [P, T*F] buffer - each step writes its own disjoint range, so there
    is no WAR edge at all.
  - Small blocks at both ends shrink pipeline fill (first timestep is
    one transfer per ring; u_0 = x_0 skips the first op) and drain.
"""

import numpy as np

import concourse.dve_ops as dve_ops
import concourse.tile as tile
from concourse import bacc, mybir
from concourse.bass_utils import run_bass_kernel_spmd
from concourse.dve_spec import Spec, Src0, Src1, C0, Zero, select, lower
from concourse.dve_uop import DveOpSpec

T = 64
B = 32
N = 8192
NCORES = 8
PERCORE = (B * N) // NCORES  # 32768 element chains per core
P = 128                      # SBUF partitions
F = PERCORE // P             # 256 elements per partition
H = F // 2                   # half-width for the two interleaved streams

V_TH = 1.0

# z/output block sizes: small at the edges to cut pipeline fill/drain
BLOCKS = [1, 2, 3, 4, 6] + [8] * 5 + [4, 2, 1, 1]
assert sum(BLOCKS) == T

# input transfer schedule: tiny steps up front so the chain starts the
# moment one timestep lands, then few fat chunks (16 steps = 16 KiB
# contiguous per partition per ring-half) for max DMA efficiency and a
# minimal semaphore count (the epilogue zeroes every sem one by one)
IN_BLOCKS = [1, 1, 2, 4, 8, 8, 8, 8, 8, 8, 4, 2, 2]
assert sum(IN_BLOCKS) == T

_NC_CACHE = {}


def _register_op(name, spec):
    for op in dve_ops.OPS:
        if op.name == name:
            return op
    row = dve_ops._CUSTOM_DVE_ROW_BASE + len(dve_ops.OPS)
    rd1 = dve_ops.has_src1(spec)
    shas = {}
    for ver in ("v3", "v4"):
        s = DveOpSpec(name=name, opcode=row, uops=lower(spec, ver=ver), rd1_en=rd1)
        shas[ver] = s.sha(ver)
    op = dve_ops.DveOp(name, spec, subdim=False, uops_sha=shas)
    dve_ops.OPS.append(op)
    dve_ops._SUB_OPCODE_FOR_NAME[name] = row
    dve_ops.CUSTOM_DVE_SPECS[name] = spec
    return op


def register_if_step():
    """Runtime-register the fused IF-step custom DVE op:
        out = select(in0 < s0, in0, 0) + in1
    i.e. hard-reset the carried potential where it crossed threshold,
    then charge with the new input -- the whole per-step recurrence in
    one Vector instruction. Uses the standard extension point
    (dve_ops.OPS + sub-opcode registry); the uop table is generated
    per-NEFF, so no firmware change is involved."""
    return _register_op(
        "IF_STEP_ANT",
        Spec(
            body=select(Src0 < C0, Src0, Zero) + Src1,
            reference=lambda in0, in1, s0, s1, imm2: np.where(
                in0 < s0, in0, 0.0
            ).astype(np.float32)
            + in1,
        ),
    )


def register_if_last():
    """Final-step fusion: z_63 = [u_63 < 1] computed straight from
    (u_62, x_63) in one DVE op -- u_63 itself is never stored and the
    final block needs no separate z pass."""
    body = (select(Src0 < C0, Src0, Zero) + Src1) < C0
    return _register_op(
        "IF_LAST_ANT",
        Spec(
            body=body,
            reference=lambda in0, in1, s0, s1, imm2: (
                (np.where(in0 < s0, in0, 0.0).astype(np.float32) + in1) < s0
            ).astype(np.float32),
        ),
    )


IF_STEP = register_if_step()
IF_LAST = register_if_last()


def build_nc(blocks=None, in_blocks=None, nstreams=2, tail_dve_steps=3,
             in_rings="sp_gp", out_rings="act", early_thirds=False):
    blocks = list(BLOCKS if blocks is None else blocks)
    in_blocks = list(IN_BLOCKS if in_blocks is None else in_blocks)
    # Bacc (not raw Bass): its compile() splits multi-wait sync conditions
    # into nop/event-semaphore prefixes — walrus accepts at most one sync
    # wait per hardware instruction.
    nc = bacc.Bacc("TRN2", target_bir_lowering=False, debug=False)
    x = nc.dram_tensor("x", [P, T, F], mybir.dt.float32, kind="ExternalInput").ap()
    y = nc.dram_tensor("y", [P, T, F], mybir.dt.uint8, kind="ExternalOutput").ap()

    with tile.TileContext(nc) as tc:
        with (
            tc.tile_pool(name="xf", bufs=1) as xfpool,
            tc.tile_pool(name="ubf", bufs=1) as ubfpool,
            tc.tile_pool(name="stf", bufs=1) as stfpool,
        ):
            # Issue ALL input dma_starts first: input tiles have no deps, so
            # the input DGE rings fill their descriptor queues up front and
            # stream ahead of the chain. Inputs get TWO dedicated rings
            # (SP + GpSimd); outputs go on the ACT ring, so they stream out
            # during the chain instead of queueing behind 8 MiB of input.
            xflat = xfpool.tile([P, T * F], mybir.dt.float32)
            t0 = 0
            for bi, tb in enumerate(in_blocks):
                xt = xflat[:, t0 * F:(t0 + tb) * F]
                ring_b = nc.gpsimd if in_rings == "sp_gp" else nc.scalar
                # NOTE: every ring must deliver in strict chain-need order.
                # Prefetching late chunks early, or handing early chunks to
                # a third ring while another ring runs ahead, both measured
                # 1.5-6us WORSE: HBM is the scarce resource even during the
                # ramp, and out-of-order transfers starve the fill-critical
                # chunks the chain is stalled on.
                if tb <= 2:
                    # head: one full-width transfer per step, alternating
                    # rings, so the chain starts as soon as step 0 lands
                    for ti in range(tb):
                        eng = nc.sync if (t0 + ti) % 2 == 0 else ring_b
                        eng.dma_start(xt[:, ti * F:(ti + 1) * F], x[:, t0 + ti, :])
                elif early_thirds and tb <= 4:
                    # fill region: all three rings collaborate per chunk
                    s1, s2 = tb // 3 + 1, 2 * (tb // 3) + 1
                    nc.sync.dma_start(xt[:, :s1 * F], x[:, t0:t0 + s1, :])
                    ring_b.dma_start(
                        xt[:, s1 * F:s2 * F], x[:, t0 + s1:t0 + s2, :])
                    third = nc.scalar if in_rings == "sp_gp" else nc.gpsimd
                    third.dma_start(
                        xt[:, s2 * F:], x[:, t0 + s2:t0 + tb, :])
                else:
                    # bulk: split the chunk between the rings by time, the
                    # earlier half on the ring that finishes first
                    th = tb // 2
                    nc.sync.dma_start(xt[:, :th * F], x[:, t0:t0 + th, :])
                    ring_b.dma_start(xt[:, th * F:], x[:, t0 + th:t0 + tb, :])
                t0 += tb

            # flat spike buffer: disjoint per-block ranges, so the tail z
            # ops on the DVE never wait on an output DMA (WAR)
            stflat = stfpool.tile([P, T * F], mybir.dt.uint8)
            # one flat u-history buffer: every step writes its own disjoint
            # range, so there is NO ub WAR edge at all
            ubflat = ubfpool.tile([P, T * F], mybir.dt.float32)

            sw = F // nstreams  # stream width
            # output transfers are MERGED across z blocks mid-run (they
            # interleave into HBM gaps while input streams), and split
            # fine across ALTERNATING idle rings at the tail: after the
            # input finishes, the drain is ring-rate-bound, so two rings
            # halve it. The epilogue zeroes a fixed 250-sem space no
            # matter how many DMAs run, so extra transfers only cost
            # their ~0.6us enqueue.
            bounds = [1, 3, 16, 32, 48, 56, 60, 62, 63, 64]
            block_ends, run = set(), 0
            for tb in blocks:
                run += tb
                block_ends.add(run)
            assert set(bounds) <= block_ends and bounds[-1] == T
            out_lo = 0
            t0 = 0
            for bi, tb in enumerate(blocks):
                for ti in range(tb):
                    t = t0 + ti
                    if t == 0:
                        # v_0 = 0, so u_0 = x_0: the spike pass and the
                        # t=1 chain op read the x tile directly
                        continue
                    src = xflat if t == 1 else ubflat
                    for h in range(nstreams):
                        lo = t * F + h * sw
                        plo = (t - 1) * F + h * sw
                        if t == T - 1:
                            # fused final step: write z directly as uint8
                            nc.vector._custom_dve(
                                IF_LAST,
                                out=stflat[:, lo:lo + sw],
                                in0=src[:, plo:plo + sw],
                                in1=xflat[:, lo:lo + sw],
                                s0=V_TH,
                            )
                        else:
                            nc.vector._custom_dve(
                                IF_STEP,
                                out=ubflat[:, lo:lo + sw],
                                in0=src[:, plo:plo + sw],
                                in1=xflat[:, lo:lo + sw],
                                s0=V_TH,
                            )
                zt0, ztb = t0, tb
                if t0 + tb == T:
                    ztb -= 1  # final step's z came from IF_LAST
                if ztb > 0:
                    st = stflat[:, zt0 * F:(zt0 + ztb) * F]
                    usrc = xflat if zt0 == 0 and ztb == 1 else ubflat
                    usl = usrc[:, zt0 * F:(zt0 + ztb) * F]
                    if zt0 + ztb > T - tail_dve_steps:
                        # final blocks: the ACT hop (engine handoff + 222-cycle
                        # SBUF latency) is a pure tail; one DVE op computes
                        # z = (u < V_TH) directly instead
                        nc.vector.tensor_scalar(
                            st[:], usl, V_TH, None, mybir.AluOpType.is_lt
                        )
                    else:
                        # One ACT pass: z = Sign(V_TH - u) in {-1,0,1}; the
                        # uint8 store saturates to {0,1}, so z = [u < V_TH]
                        # = 1 - s exactly (u == V_TH -> Sign(0) = 0 ->
                        # spike). Host flips.
                        nc.scalar.activation(
                            st[:], usl, mybir.ActivationFunctionType.Sign,
                            bias=V_TH, scale=-1.0,
                        )
                # outputs: mid-run groups on the ACT ring (no input ahead
                # of them); tail groups alternate SP/ACT so both HWDGE
                # rings drain the last spikes in parallel
                if t0 + tb in bounds:
                    end = t0 + tb
                    if end <= 48:
                        oeng = nc.scalar
                    else:
                        oeng = nc.sync if bounds.index(end) % 2 == 1 else nc.scalar
                    oeng.dma_start(
                        y[:, out_lo:end, :],
                        stflat[:, out_lo * F:end * F],
                    )
                    out_lo = end
                t0 += tb
    nc.compile()
    return nc


def _get_nc():
    if "nc" not in _NC_CACHE:
        _NC_CACHE["nc"] = build_nc()
    return _NC_CACHE["nc"]


def run_sharded(x_seq, trace=False, nc=None, **kwargs):
    if nc is None:
        nc = _get_nc()
    x2 = np.asarray(x_seq, dtype=np.float32).reshape(T, B * N)
    in_maps = []
    for c in range(NCORES):
        # core slab [T, PERCORE] -> [P, T, F]: partition-major, time
        # contiguous per partition so each block is one fat descriptor
        xc = x2[:, c * PERCORE:(c + 1) * PERCORE].reshape(T, P, F)
        in_maps.append({"x": np.ascontiguousarray(xc.transpose(1, 0, 2))})
    # A cold device occasionally reports NRT_EXEC_UNIT_UNRECOVERABLE on the
    # first execute and recovers on the next attempt; retry a couple times.
    for attempt in range(3):
        try:
            res = run_bass_kernel_spmd(
                nc, in_maps, list(range(NCORES)), trace=trace, **kwargs
            )
            break
        except Exception:  # jax.errors.JaxRuntimeError and friends
            if attempt == 2:
                raise
            import time
            time.sleep(2.0)
    out = np.empty((T, B * N), dtype=np.float32)
    for c in range(NCORES):
        zc = np.asarray(res.results[c]["y"])          # [P, T, F] uint8, z = 1-s
        r = zc.transpose(1, 0, 2).reshape(T, PERCORE)
        out[:, c * PERCORE:(c + 1) * PERCORE] = 1 - r
    return out.reshape(T, B, N), res


def kernel(x_seq):
    out, _ = run_sharded(x_seq)
    return out
